# revision 1
# baseline (speedup 1.0000x reference)
"""Trainium2 Bass kernel for the EdgeMask problem.

Computes, for h (B,T,N,d), I_full (B,T,N,N), MLP params W1 (2d,hid) b1 (hid,)
W2 (hid,) b2 (1,):
    li = h @ W1[:d]; lj = h @ W1[d:]
    hid = relu(li[:,:,:,None,:] + lj[:,:,None,:,:] + b1)
    M = sigmoid(hid @ W2 + b2);  I_sparse = I_full * M
Returns (I_sparse, M).

Sharding: data-parallel over B across 8 NeuronCores (B=8), no collectives.

Per-core layout (per (t) slice, N=128 nodes, d=128, K=32 hidden):
  - hT = h[t].T via PE transpose (d on partitions)
  - liT/ljT via col-tiled PE matmuls with W1a/W1b as stationary operands.
    Partition stacking p = 32*gp + k (4 replicas of the 32 hidden units).
    "Group" g covers rows i in {g, g+32, g+64, g+96} (i = g + 32*gp).
      R[32gp+k, j]  = lj[j,k] + b1[k]        (replicated 4x, ACT adds b1)
      S[32gp+k, g]  = li[g+32gp, k]          (li "stack", fp32)
  - Pointwise (the N^2*K hot loop): for each group g one fused op
      hid_g = max(R + S[:,g], 0)   -- DVE tensor_scalar(add,max) / ACT Relu+bias
  - Reduce over k via PE: lhsT = blockdiag(W2 x4) (128,4), col-tiled 4-way,
    rhs = 4 groups' hid packed into (128,512):
      logits'[i=4w+c+32m, j] at PSUM[32q+m, 128c+j]  (w = 4*half + q)
  - Compact 2 PSUM banks -> dense (128,128) via DMA, sigmoid(+b2) on ACT,
    I_full * M on DVE, DMA out.
"""

import functools

import numpy as np

import concourse.bass as bass
import concourse.mybir as mybir
import concourse.tile as tile
from concourse import bacc

F32 = mybir.dt.float32
F16 = mybir.dt.float16

B = 8
T = 32
N = 128
D = 128
K = 32  # hidden
NCORES = 8

AFT = mybir.ActivationFunctionType
ALU = mybir.AluOpType

# dtype of the hid (pointwise+reduce) path: F16 -> DVE 4x mode, F32 exact
HID_DT = F16
HID_NP = np.float16 if HID_DT == F16 else np.float32

# pointwise split: first ACT_SHARE_G groups on ACT, last POOL_SHARE_G on
# GPSIMD, rest on DVE
ACT_SHARE_G = 3
POOL_SHARE_G = 10
HID_BUFS = 24
SIG_DENSE = False
PW_SPREAD = False
MASK_ON_POOL = False
IO_BUFS = 4
OUT_BUFS = 3
RS_BUFS = 3
LILJ_BUFS = 2
MP_BUFS = 2
S_ON_ACT = False
MEXIT_SPLIT = False


def _pw_engine(g):
    if PW_SPREAD:
        # interleave: pool every 3rd, act sprinkled, rest dve
        w, c = divmod(g, 4)
        if c == 3 and w >= 8 - POOL_SHARE_G // 4 * 4:
            pass
        seq = (["dve"] * (K - ACT_SHARE_G - POOL_SHARE_G)
               + ["pool"] * POOL_SHARE_G + ["act"] * ACT_SHARE_G)
        # round-robin-ish deterministic shuffle
        return seq[(g * 7) % K]
    if g < ACT_SHARE_G:
        return "act"
    if g >= K - POOL_SHARE_G:
        return "pool"
    return "dve"


def _build(t_slices: int = T, skip=()):
    nc = bacc.Bacc(
        "TRN2", target_bir_lowering=False, debug=False, num_devices=NCORES
    )

    ht_d = nc.dram_tensor("ht", [D, t_slices * N], HID_DT, kind="ExternalInput")
    i_d = nc.dram_tensor("ifull", [t_slices, N, N], F32, kind="ExternalInput")
    w1a_d = nc.dram_tensor("w1a", [D, K], HID_DT, kind="ExternalInput")
    w1b_d = nc.dram_tensor("w1b", [D, K], HID_DT, kind="ExternalInput")
    b1t_d = nc.dram_tensor("b1t", [128, 1], F32, kind="ExternalInput")
    wd_d = nc.dram_tensor("wd", [128, 32], HID_DT, kind="ExternalInput")
    b2t_d = nc.dram_tensor("b2t", [128, 1], F32, kind="ExternalInput")
    perm_d = nc.dram_tensor("perm", [128, 8 * 128], HID_DT, kind="ExternalInput")

    # merged output: [..., 0:N] = M, [..., N:2N] = I_sparse (one store per slice)
    mi_d = nc.dram_tensor("mi", [t_slices, N, 2 * N], F32, kind="ExternalOutput")

    with tile.TileContext(nc) as tc:
        with (
            tc.tile_pool(name="const", bufs=1) as cpool,
            tc.tile_pool(name="hin", bufs=4) as hpool,
            tc.tile_pool(name="hts", bufs=3) as htpool,
            tc.tile_pool(name="rs", bufs=RS_BUFS) as rspool,
            tc.tile_pool(name="hid", bufs=HID_BUFS) as hidpool,
            tc.tile_pool(name="io", bufs=IO_BUFS) as iopool,
            tc.tile_pool(name="outp", bufs=OUT_BUFS) as opool,
            tc.tile_pool(name="psum", bufs=2, space="PSUM") as ppool,
        ):
            w1a_sb = cpool.tile([D, K], HID_DT)
            nc.sync.dma_start(w1a_sb[:], w1a_d[:])
            w1b_sb = cpool.tile([D, K], HID_DT)
            nc.sync.dma_start(w1b_sb[:], w1b_d[:])
            b1t_sb = cpool.tile([128, 1], F32)
            nc.sync.dma_start(b1t_sb[:], b1t_d[:])
            wd_sb = cpool.tile([128, 32], HID_DT)
            nc.sync.dma_start(wd_sb[:], wd_d[:])
            b2t_sb = cpool.tile([128, 1], F32)
            nc.sync.dma_start(b2t_sb[:], b2t_d[:])
            perm_sb = cpool.tile([128, 8 * 128], HID_DT)
            nc.sync.dma_start(perm_sb[:], perm_d[:])
            # all slices' hT, chunked so slice 0 can start early
            htall_sb = cpool.tile([D, t_slices * N], HID_DT)
            n_chunks = min(8, t_slices)
            chunk = t_slices * N // n_chunks
            for ci in range(n_chunks):
                nc.sync.dma_start(
                    htall_sb[:, ci * chunk : (ci + 1) * chunk],
                    ht_d[:, ci * chunk : (ci + 1) * chunk],
                )

            for t in range(t_slices):
                ht_sb = htall_sb[:, t * N : (t + 1) * N]

                # ---- liT / ljT, col-tiled (4 concurrent 32-col groups) ----
                lilj_ps = ppool.tile([128, N + K], F32, tag="lilj", bufs=LILJ_BUFS)
                for gp in range(4):
                    # ljT replicated: out[32gp+k, j] = lj[j, k]
                    nc.tensor.matmul(
                        lilj_ps[32 * gp : 32 * gp + 32, 0:N],
                        w1b_sb[:],
                        ht_sb,
                        tile_position=(0, 32 * gp),
                        skip_group_check=True,
                    )
                for gp in range(4):
                    # li stack: out[32gp+k, g] = li[g+32gp, k]
                    nc.tensor.matmul(
                        lilj_ps[32 * gp : 32 * gp + 32, N : N + K],
                        w1a_sb[:],
                        ht_sb[:, 32 * gp : 32 * gp + 32],
                        tile_position=(0, 32 * gp),
                        skip_group_check=True,
                    )

                # R = ljT_rep + b1 (cast to HID_DT); S = li stack (fp32)
                r_sb = rspool.tile([128, N], HID_DT, tag="r")
                nc.scalar.activation(
                    r_sb[:], lilj_ps[:, 0:N], AFT.Identity, bias=b1t_sb[:, 0:1]
                )
                s_sb = rspool.tile([128, K], F32, tag="s")
                if S_ON_ACT:
                    nc.scalar.copy(s_sb[:], lilj_ps[:, N : N + K])
                else:
                    nc.vector.tensor_copy(s_sb[:], lilj_ps[:, N : N + K])

                # ---- pointwise: hid_g = relu(R + S[:, g]) ----
                hbufs = [
                    hidpool.tile([128, 4 * N], HID_DT, tag="hid", name=f"hb{w}")
                    for w in range(8)
                ]
                for g in range(K):
                    w, c = divmod(g, 4)
                    dst = hbufs[w][:, c * N : (c + 1) * N]
                    eng = _pw_engine(g)
                    if eng == "act":
                        nc.scalar.activation(
                            dst, r_sb[:], AFT.Relu, bias=s_sb[:, g : g + 1]
                        )
                    elif eng == "pool":
                        nc.gpsimd.tensor_scalar(
                            dst, r_sb[:], s_sb[:, g : g + 1], 0.0, ALU.add, ALU.max
                        )
                    else:
                        nc.vector.tensor_scalar(
                            dst, r_sb[:], s_sb[:, g : g + 1], 0.0, ALU.add, ALU.max
                        )

                # ---- reduce over k on PE (col-tiled, 2 waves of 4) ----
                l_ps = [
                    ppool.tile([128, 4 * N], F32, tag="l0", name="l0"),
                    ppool.tile([128, 4 * N], F32, tag="l1", name="l1"),
                ]
                for w in range(8):
                    half, q = divmod(w, 4)
                    nc.tensor.matmul(
                        l_ps[half][32 * q : 32 * q + 32, :],
                        wd_sb[:],
                        hbufs[w][:],
                        tile_position=(0, 32 * q),
                    )

                # ---- PSUM exits (cast fp16) to SBUF ----
                # SIG_DENSE: raw-logit copies here, sigmoid after the permute.
                # else: sigmoid(+b2) applied here (sparse), permute carries M.
                lsp = [
                    opool.tile([128, 4 * N], HID_DT, tag="lsp0", name="lsp0"),
                    opool.tile([128, 4 * N], HID_DT, tag="lsp1", name="lsp1"),
                ]
                if SIG_DENSE:
                    nc.scalar.copy(lsp[0][:], l_ps[0][:])
                    nc.vector.tensor_copy(lsp[1][:], l_ps[1][:])
                else:
                    for half in range(2):
                        nc.scalar.activation(
                            lsp[half][:], l_ps[half][:], AFT.Sigmoid,
                            bias=b2t_sb[:, 0:1],
                        )

                # ---- un-permute logits on PE: 8 accumulating matmuls with 0/1
                # permutation matrices; P_b[32q+m, 32m+16h+4q+c] = 1, b=4h+c
                mp_ps = ppool.tile([128, N], F32, tag="mp", bufs=MP_BUFS)
                for b in range(8):
                    h, c = divmod(b, 4)
                    nc.tensor.matmul(
                        mp_ps[:],
                        perm_sb[:, 128 * b : 128 * (b + 1)],
                        lsp[h][:, 128 * c : 128 * (c + 1)],
                        start=(b == 0),
                        stop=(b == 7),
                    )
                # dense-PSUM exit; M lands in mi[:, 0:N]
                mi_sb = opool.tile([128, 2 * N], F32, tag="mi")
                if SIG_DENSE:
                    nc.scalar.activation(
                        mi_sb[:, 0:N], mp_ps[:], AFT.Sigmoid, bias=b2t_sb[:, 0:1]
                    )
                elif MEXIT_SPLIT:
                    nc.vector.tensor_copy(mi_sb[:, 0 : N // 2], mp_ps[:, 0 : N // 2])
                    nc.scalar.copy(mi_sb[:, N // 2 : N], mp_ps[:, N // 2 : N])
                else:
                    nc.vector.tensor_copy(mi_sb[:, 0:N], mp_ps[:])
                i_sb = iopool.tile([N, N], F32, tag="i")
                nc.sync.dma_start(i_sb[:], i_d[t, :, :])
                if MASK_ON_POOL:
                    nc.gpsimd.tensor_tensor(
                        mi_sb[:, N : 2 * N], i_sb[:], mi_sb[:, 0:N], ALU.mult
                    )
                else:
                    nc.vector.tensor_tensor(
                        mi_sb[:, N : 2 * N], i_sb[:], mi_sb[:, 0:N], ALU.mult
                    )
                nc.sync.dma_start(mi_d[t, :, :], mi_sb[:])

    nc.compile()
    return nc


def make_aux_inputs(W1, b1, W2, b2):
    W1 = np.asarray(W1)
    w1a = np.ascontiguousarray(W1[:D]).astype(HID_NP)
    w1b = np.ascontiguousarray(W1[D:]).astype(HID_NP)
    b1t = np.ascontiguousarray(np.tile(np.asarray(b1, np.float32), 4).reshape(128, 1))
    # col m carries W2 at partition-block (m % 4): every PSUM output row of the
    # reduce matmul is then a valid (replicated) logits row
    wd = np.zeros((128, 32), HID_NP)
    for m in range(32):
        gp = m % 4
        wd[32 * gp : 32 * gp + 32, m] = np.asarray(W2)
    b2t = np.full((128, 1), np.asarray(b2, np.float32)[0], np.float32)
    perm = np.zeros((8, 128, 128), np.float32)
    for h in range(2):
        for c in range(4):
            for q in range(4):
                for m in range(4):
                    perm[4 * h + c, 32 * q + m, 32 * m + 16 * h + 4 * q + c] = 1.0
    perm = np.ascontiguousarray(np.concatenate(list(perm), axis=1)).astype(HID_NP)
    return {
        "perm": perm,
        "w1a": w1a,
        "w1b": w1b,
        "b1t": b1t,
        "wd": wd,
        "b2t": b2t,
    }


TRACE = False
LAST_RESULTS = None


@functools.lru_cache(maxsize=1)
def _built_nc():
    return _build(T)


def kernel(**inputs):
    from concourse.bass_utils import run_bass_kernel_spmd

    h = np.asarray(inputs["h"])
    # (B, T, N, D) -> (B, D, T*N) so one DMA per core loads all hT with 8KB runs
    ht = np.ascontiguousarray(np.transpose(h, (0, 3, 1, 2)).reshape(B, D, -1)).astype(
        HID_NP
    )
    ifull = np.ascontiguousarray(np.asarray(inputs["I_full"], np.float32))
    aux = make_aux_inputs(inputs["W1"], inputs["b1"], inputs["W2"], inputs["b2"])

    nc = _built_nc()
    in_maps = [{"ht": ht[c], "ifull": ifull[c], **aux} for c in range(NCORES)]
    res = run_bass_kernel_spmd(
        nc, in_maps, core_ids=list(range(NCORES)), trace=TRACE
    )
    global LAST_RESULTS
    LAST_RESULTS = res
    mi = np.stack([res.results[c]["mi"] for c in range(NCORES)])
    return np.ascontiguousarray(mi[..., N:]), np.ascontiguousarray(mi[..., :N])



# revision 5
# speedup vs baseline: 1.0366x; 1.0366x over previous
"""Trainium2 Bass kernel for the EdgeMask problem.

Computes, for h (B,T,N,d), I_full (B,T,N,N), MLP params W1 (2d,hid) b1 (hid,)
W2 (hid,) b2 (1,):
    li = h @ W1[:d]; lj = h @ W1[d:]
    hid = relu(li[:,:,:,None,:] + lj[:,:,None,:,:] + b1)
    M = sigmoid(hid @ W2 + b2);  I_sparse = I_full * M
Returns (I_sparse, M).

Sharding: data-parallel over B across 8 NeuronCores (B=8), no collectives.

Per-core pipeline (per t slice, N=128, d=128, K=32 hidden):
  - PE: ljT-replicated (one matmul, W1b pre-replicated 4x in cols) and a
    li "stack" S[32*gp+k, g] = li[g+32*gp, k] (+b1 via a rank-1 accumulate
    matmul), both in one PSUM tile.
  - Pointwise hid_g = relu(R + S[:, g]) as 32 fp16 tensor_scalar ops
    ([128,128] each, bias read straight from PSUM), split across
    DVE / ACT / GPSIMD.
  - Reduce over k on PE: per column strip q, two accumulating matmuls with
    zero-padded block-diag W2 stationaries (phase h in {0,1}) consume the
    8 hid buffers; result is a COMPACT [128, 512] PSUM tile whose rows
    32q+4h+m hold logits for i = 32m+8q+4h+c at free chunk c.
  - ACT applies sigmoid(+b2) on the full [128,512] tile (junk rows incl.),
    DVE multiplies with the host-prepermuted I tile, both halves land in
    one [128, 1024] fp16 tile, stored permuted; the host unpermutes/casts.
"""

import functools

import numpy as np

import concourse.bass as bass
import concourse.mybir as mybir
import concourse.tile as tile
from concourse import bacc

F32 = mybir.dt.float32
F16 = mybir.dt.float16

B = 8
T = 32
N = 128
D = 128
K = 32  # hidden
NCORES = 8

AFT = mybir.ActivationFunctionType
ALU = mybir.AluOpType

# pointwise split: group g -> engine. roughly DVE 19 / ACT 6 / Pool 7
PW_DVE = 19
PW_ACT = 5
PW_POOL = 8
HID_BUFS = 2
IO_BUFS = 2
OUT_BUFS = 3
R_BUFS = 3
LILJ_BUFS = 2
RED_BUFS = 2
I_BATCH = 4  # slices per I-load DMA
R_ON_POOL = False


def _pw_engine(g):
    # deterministic interleave so each engine's work spreads over the slice
    seq = (["dve"] * PW_DVE + ["act"] * PW_ACT + ["pool"] * PW_POOL)
    return seq[(g * 7) % K]


def _build(t_slices: int = T):
    nc = bacc.Bacc(
        "TRN2", target_bir_lowering=False, debug=False, num_devices=NCORES
    )

    ht_d = nc.dram_tensor("ht", [D, t_slices * N], F16, kind="ExternalInput")
    ip_d = nc.dram_tensor("ip", [t_slices, N, 4 * N], F16, kind="ExternalInput")
    w1brep_d = nc.dram_tensor("w1brep", [D, 128], F16, kind="ExternalInput")
    w1a_d = nc.dram_tensor("w1a", [D, K], F16, kind="ExternalInput")
    b1col_d = nc.dram_tensor("b1col", [1, 128], F16, kind="ExternalInput")
    ones_d = nc.dram_tensor("ones32", [1, K], F16, kind="ExternalInput")
    wda_d = nc.dram_tensor("wda", [128, K], F16, kind="ExternalInput")
    wdb_d = nc.dram_tensor("wdb", [128, K], F16, kind="ExternalInput")
    b2col_d = nc.dram_tensor("b2col", [128, 1], F32, kind="ExternalInput")

    # permuted merged output: [..., 0:512] = M, [..., 512:1024] = I_sparse
    mi_d = nc.dram_tensor("mi", [t_slices, N, 8 * N], F16, kind="ExternalOutput")

    with tile.TileContext(nc) as tc:
        with (
            tc.tile_pool(name="const", bufs=1) as cpool,
            tc.tile_pool(name="rsb", bufs=R_BUFS) as rpool,
            tc.tile_pool(name="hid", bufs=HID_BUFS) as hidpool,
            tc.tile_pool(name="io", bufs=IO_BUFS) as iopool,
            tc.tile_pool(name="outp", bufs=OUT_BUFS) as opool,
            tc.tile_pool(name="psum", bufs=1, space="PSUM") as ppool,
        ):
            w1brep_sb = cpool.tile([D, 128], F16)
            nc.sync.dma_start(w1brep_sb[:], w1brep_d[:])
            w1a_sb = cpool.tile([D, K], F16)
            nc.sync.dma_start(w1a_sb[:], w1a_d[:])
            b1col_sb = cpool.tile([1, 128], F16)
            nc.sync.dma_start(b1col_sb[:], b1col_d[:])
            ones_sb = cpool.tile([1, K], F16)
            nc.sync.dma_start(ones_sb[:], ones_d[:])
            wda_sb = cpool.tile([128, K], F16)
            nc.sync.dma_start(wda_sb[:], wda_d[:])
            wdb_sb = cpool.tile([128, K], F16)
            nc.sync.dma_start(wdb_sb[:], wdb_d[:])
            b2col_sb = cpool.tile([128, 1], F32)
            nc.sync.dma_start(b2col_sb[:], b2col_d[:])

            # all slices' hT, chunked so slice 0 can start early
            htall_sb = cpool.tile([D, t_slices * N], F16)
            n_chunks = min(8, t_slices)
            chunk = t_slices * N // n_chunks
            for ci in range(n_chunks):
                nc.sync.dma_start(
                    htall_sb[:, ci * chunk : (ci + 1) * chunk],
                    ht_d[:, ci * chunk : (ci + 1) * chunk],
                )

            for t in range(t_slices):
                ht_sb = htall_sb[:, t * N : (t + 1) * N]

                # ---- R (ljT replicated) + S (li stack + b1) in one PSUM ----
                lilj_ps = ppool.tile(
                    [128, N + K], F32, tag="lilj", bufs=LILJ_BUFS
                )
                nc.tensor.matmul(lilj_ps[:, 0:N], w1brep_sb[:], ht_sb)
                for gp in range(4):
                    nc.tensor.matmul(
                        lilj_ps[32 * gp : 32 * gp + 32, N : N + K],
                        w1a_sb[:],
                        ht_sb[:, 32 * gp : 32 * gp + 32],
                        tile_position=(0, 32 * gp),
                        start=True,
                        stop=False,
                        skip_group_check=True,
                    )
                nc.tensor.matmul(
                    lilj_ps[:, N : N + K],
                    b1col_sb[:],
                    ones_sb[:],
                    start=False,
                    stop=True,
                    skip_group_check=True,
                )

                # ---- R -> SBUF fp16, S' -> SBUF f32 ----
                r_sb = rpool.tile([128, N], F16, tag="r")
                if R_ON_POOL:
                    nc.gpsimd.tensor_copy(r_sb[:], lilj_ps[:, 0:N])
                else:
                    nc.scalar.copy(r_sb[:], lilj_ps[:, 0:N])
                s_sb = rpool.tile([128, K], F32, tag="s")
                nc.vector.tensor_copy(s_sb[:], lilj_ps[:, N : N + K])

                # ---- pointwise: hid_g = relu(R + S[:, g]) ----
                # hbuf (h, q) holds groups g = 8q+4h+c at chunk c
                hbufs = [
                    hidpool.tile([128, 4 * N], F16, tag=f"hb{w}", name=f"hb{w}")
                    for w in range(8)
                ]
                for g in range(K):
                    q, rem = divmod(g, 8)
                    h, c = divmod(rem, 4)
                    dst = hbufs[4 * h + q][:, c * N : (c + 1) * N]
                    s_col = s_sb[:, g : g + 1]
                    eng = _pw_engine(g)
                    if eng == "act":
                        nc.scalar.activation(
                            dst, r_sb[:], AFT.Relu, bias=s_col
                        )
                    elif eng == "pool":
                        nc.gpsimd.tensor_scalar(
                            dst, r_sb[:], s_col, 0.0, ALU.add, ALU.max
                        )
                    else:
                        nc.vector.tensor_scalar(
                            dst, r_sb[:], s_col, 0.0, ALU.add, ALU.max
                        )

                # ---- reduce over k on PE: compact [128, 512] psum ----
                red_ps = ppool.tile([128, 4 * N], F32, tag="red", bufs=RED_BUFS)
                for q in range(4):
                    nc.tensor.matmul(
                        red_ps[32 * q : 32 * q + 32, :],
                        wda_sb[:],
                        hbufs[q][:],
                        tile_position=(0, 32 * q),
                        start=True,
                        stop=False,
                        skip_group_check=True,
                    )
                    nc.tensor.matmul(
                        red_ps[32 * q : 32 * q + 32, :],
                        wdb_sb[:],
                        hbufs[4 + q][:],
                        tile_position=(0, 32 * q),
                        start=False,
                        stop=True,
                        skip_group_check=True,
                    )

                # ---- sigmoid + mask multiply, both into one fp16 tile ----
                mi_sb = opool.tile([128, 8 * N], F16, tag="mi")
                nc.scalar.activation(
                    mi_sb[:, 0 : 4 * N], red_ps[:], AFT.Sigmoid, bias=b2col_sb[:, 0:1]
                )
                if t % I_BATCH == 0:
                    ip_sb = iopool.tile(
                        [128, I_BATCH * 4 * N], F16, tag="ip", name="ip"
                    )
                    nc.sync.dma_start(
                        ip_sb[:],
                        ip_d[t : t + I_BATCH].rearrange("t p f -> p t f"),
                    )
                nc.vector.tensor_tensor(
                    mi_sb[:, 4 * N : 8 * N],
                    mi_sb[:, 0 : 4 * N],
                    ip_sb[:, (t % I_BATCH) * 4 * N : (t % I_BATCH + 1) * 4 * N],
                    ALU.mult,
                )
                nc.sync.dma_start(mi_d[t, :, :], mi_sb[:])

    nc.compile()
    return nc


def make_aux_inputs(W1, b1, W2, b2):
    W1 = np.asarray(W1, np.float32)
    W1a = W1[:D]
    W1b = W1[D:]
    w1brep = np.zeros((D, 128), np.float16)
    for gp in range(4):
        w1brep[:, 32 * gp : 32 * gp + 32] = W1b.astype(np.float16)
    w1a = np.ascontiguousarray(W1a).astype(np.float16)
    b1col = np.tile(np.asarray(b1, np.float32), 4).reshape(1, 128).astype(np.float16)
    ones32 = np.ones((1, K), np.float16)
    # zero-padded block-diag W2 stationaries: phase h col (4h+m) has W2 at
    # partition block m
    wda = np.zeros((128, K), np.float16)
    wdb = np.zeros((128, K), np.float16)
    for m in range(4):
        wda[32 * m : 32 * m + 32, 4 * 0 + m] = np.asarray(W2, np.float16)
        wdb[32 * m : 32 * m + 32, 4 * 1 + m] = np.asarray(W2, np.float16)
    b2col = np.full((128, 1), np.asarray(b2, np.float32)[0], np.float32)
    return {
        "w1brep": w1brep,
        "w1a": w1a,
        "b1col": b1col,
        "ones32": ones32,
        "wda": wda,
        "wdb": wdb,
        "b2col": b2col,
    }


def _perm_maps():
    """row r, chunk c -> i mapping of the permuted output/I layout.

    r = 32q + 4h + m (valid for r%32 < 8), i = 32m + 8q + 4h + c.
    Returns (valid_rows[32], i_of[32, 4]) where valid_rows are the 32 rows
    holding data and i_of gives i per (row_idx, chunk).
    """
    rows = []
    i_of = []
    for q in range(4):
        for h in range(2):
            for m in range(4):
                r = 32 * q + 4 * h + m
                rows.append(r)
                i_of.append([32 * m + 8 * q + 4 * h + c for c in range(4)])
    return np.array(rows), np.array(i_of)


ROWS, I_OF = _perm_maps()


def permute_i(ifull_core):
    """I_full (T, N, N) f32 -> permuted fp16 (T, N, 4N) matching the
    on-device layout; junk rows left zero."""
    out = np.zeros((T, 128, 4 * N), np.float16)
    src = ifull_core.astype(np.float16)
    for ridx, r in enumerate(ROWS):
        for c in range(4):
            out[:, r, c * N : (c + 1) * N] = src[:, I_OF[ridx, c], :]
    return out


def unpermute(mi_core):
    """Permuted (T, N, 8N) fp16 -> (M, I_sparse) each (T, N, N) f32."""
    M = np.empty((T, N, N), np.float32)
    Isp = np.empty((T, N, N), np.float32)
    for ridx, r in enumerate(ROWS):
        for c in range(4):
            i = I_OF[ridx, c]
            M[:, i, :] = mi_core[:, r, c * N : (c + 1) * N].astype(np.float32)
            Isp[:, i, :] = mi_core[:, r, 4 * N + c * N : 4 * N + (c + 1) * N].astype(
                np.float32
            )
    return Isp, M


TRACE = False
LAST_RESULTS = None


@functools.lru_cache(maxsize=1)
def _built_nc():
    return _build(T)


def kernel(**inputs):
    from concourse.bass_utils import run_bass_kernel_spmd

    h = np.asarray(inputs["h"])
    # (B, T, N, D) -> (B, D, T*N) so one DMA per core loads all hT
    ht = np.ascontiguousarray(
        np.transpose(h, (0, 3, 1, 2)).reshape(B, D, -1)
    ).astype(np.float16)
    ifull = np.asarray(inputs["I_full"], np.float32)
    aux = make_aux_inputs(
        inputs["W1"], inputs["b1"], inputs["W2"], inputs["b2"]
    )

    nc = _built_nc()
    in_maps = [
        {"ht": ht[cc], "ip": permute_i(ifull[cc]), **aux} for cc in range(NCORES)
    ]
    res = run_bass_kernel_spmd(
        nc, in_maps, core_ids=list(range(NCORES)), trace=TRACE
    )
    global LAST_RESULTS
    LAST_RESULTS = res
    isp = np.empty((B, T, N, N), np.float32)
    m = np.empty((B, T, N, N), np.float32)
    for cc in range(NCORES):
        i_c, m_c = unpermute(res.results[cc]["mi"])
        isp[cc] = i_c
        m[cc] = m_c
    return isp, m


# revision 6
# speedup vs baseline: 1.1139x; 1.0746x over previous
"""Trainium2 Bass kernel for the EdgeMask problem.

Computes, for h (B,T,N,d), I_full (B,T,N,N), MLP params W1 (2d,hid) b1 (hid,)
W2 (hid,) b2 (1,):
    li = h @ W1[:d]; lj = h @ W1[d:]
    hid = relu(li[:,:,:,None,:] + lj[:,:,None,:,:] + b1)
    M = sigmoid(hid @ W2 + b2);  I_sparse = I_full * M
Returns (I_sparse, M).

Sharding: data-parallel over B across 8 NeuronCores (B=8), no collectives.

Per-core pipeline (per t slice, N=128, d=128, K=32 hidden):
  - PE: ljT-replicated (one matmul, W1b pre-replicated 4x in cols) and a
    li "stack" S[32*gp+k, g] = li[g+32*gp, k] (+b1 via a rank-1 accumulate
    matmul), both in one PSUM tile.
  - Pointwise hid_g = relu(R + S[:, g]) as 32 fp16 tensor_scalar ops
    ([128,128] each, bias read straight from PSUM), split across
    DVE / ACT / GPSIMD.
  - Reduce over k on PE: per column strip q, two accumulating matmuls with
    zero-padded block-diag W2 stationaries (phase h in {0,1}) consume the
    8 hid buffers; result is a COMPACT [128, 512] PSUM tile whose rows
    32q+4h+m hold logits for i = 32m+8q+4h+c at free chunk c.
  - ACT applies sigmoid(+b2) on the full [128,512] tile (junk rows incl.),
    DVE multiplies with the host-prepermuted I tile, both halves land in
    one [128, 1024] fp16 tile, stored permuted; the host unpermutes/casts.
"""

import functools

import numpy as np

import concourse.bass as bass
import concourse.mybir as mybir
import concourse.tile as tile
from concourse import bacc

F32 = mybir.dt.float32
F16 = mybir.dt.float16

B = 8
T = 32
N = 128
D = 128
K = 32  # hidden
NCORES = 8

AFT = mybir.ActivationFunctionType
ALU = mybir.AluOpType

# pointwise split: group g -> engine. roughly DVE 19 / ACT 6 / Pool 7
PW_DVE = 19
PW_ACT = 5
PW_POOL = 8
HID_BUFS = 2
IO_BUFS = 2
OUT_BUFS = 3
R_BUFS = 3
LILJ_BUFS = 2
RED_BUFS = 2
I_BATCH = 4  # slices per I-load DMA
R_ON_POOL = False


def _pw_engine(g):
    # deterministic interleave so each engine's work spreads over the slice
    seq = (["dve"] * PW_DVE + ["act"] * PW_ACT + ["pool"] * PW_POOL)
    return seq[(g * 7) % K]


def _build(t_slices: int = T):
    nc = bacc.Bacc(
        "TRN2", target_bir_lowering=False, debug=False, num_devices=NCORES
    )

    ht_d = nc.dram_tensor("ht", [D, t_slices * N], F16, kind="ExternalInput")
    ip_d = nc.dram_tensor("ip", [t_slices, N, 4 * N], F16, kind="ExternalInput")
    w1brep_d = nc.dram_tensor("w1brep", [D, 128], F16, kind="ExternalInput")
    w1a_d = nc.dram_tensor("w1a", [D, K], F16, kind="ExternalInput")
    b1col_d = nc.dram_tensor("b1col", [1, 128], F16, kind="ExternalInput")
    ones_d = nc.dram_tensor("ones32", [1, K], F16, kind="ExternalInput")
    wda_d = nc.dram_tensor("wda", [128, K], F16, kind="ExternalInput")
    wdb_d = nc.dram_tensor("wdb", [128, K], F16, kind="ExternalInput")
    b2col_d = nc.dram_tensor("b2col", [128, 1], F32, kind="ExternalInput")

    # permuted merged output: [..., 0:512] = M, [..., 512:1024] = I_sparse
    mi_d = nc.dram_tensor("mi", [t_slices, N, 8 * N], F16, kind="ExternalOutput")

    with tile.TileContext(nc) as tc:
        with (
            tc.tile_pool(name="const", bufs=1) as cpool,
            tc.tile_pool(name="rsb", bufs=R_BUFS) as rpool,
            tc.tile_pool(name="hid", bufs=HID_BUFS) as hidpool,
            tc.tile_pool(name="io", bufs=IO_BUFS) as iopool,
            tc.tile_pool(name="outp", bufs=OUT_BUFS) as opool,
            tc.tile_pool(name="psum", bufs=1, space="PSUM") as ppool,
        ):
            w1brep_sb = cpool.tile([D, 128], F16)
            nc.sync.dma_start(w1brep_sb[:], w1brep_d[:])
            w1a_sb = cpool.tile([D, K], F16)
            nc.sync.dma_start(w1a_sb[:], w1a_d[:])
            b1col_sb = cpool.tile([1, 128], F16)
            nc.sync.dma_start(b1col_sb[:], b1col_d[:])
            ones_sb = cpool.tile([1, K], F16)
            nc.sync.dma_start(ones_sb[:], ones_d[:])
            wda_sb = cpool.tile([128, K], F16)
            nc.sync.dma_start(wda_sb[:], wda_d[:])
            wdb_sb = cpool.tile([128, K], F16)
            nc.sync.dma_start(wdb_sb[:], wdb_d[:])
            b2col_sb = cpool.tile([128, 1], F32)
            nc.sync.dma_start(b2col_sb[:], b2col_d[:])

            # all slices' hT, chunked so slice 0 can start early
            htall_sb = cpool.tile([D, t_slices * N], F16)
            n_chunks = min(8, t_slices)
            chunk = t_slices * N // n_chunks
            for ci in range(n_chunks):
                nc.sync.dma_start(
                    htall_sb[:, ci * chunk : (ci + 1) * chunk],
                    ht_d[:, ci * chunk : (ci + 1) * chunk],
                )

            lilj_tiles = {}
            rs_tiles = {}
            ip_tiles = {}

            def stage_a(t):
                ht_sb = htall_sb[:, t * N : (t + 1) * N]
                lilj_ps = ppool.tile(
                    [128, N + K], F32, tag="lilj", bufs=LILJ_BUFS, name="lilj"
                )
                nc.tensor.matmul(lilj_ps[:, 0:N], w1brep_sb[:], ht_sb)
                for gp in range(4):
                    nc.tensor.matmul(
                        lilj_ps[32 * gp : 32 * gp + 32, N : N + K],
                        w1a_sb[:],
                        ht_sb[:, 32 * gp : 32 * gp + 32],
                        tile_position=(0, 32 * gp),
                        start=True,
                        stop=False,
                        skip_group_check=True,
                    )
                nc.tensor.matmul(
                    lilj_ps[:, N : N + K],
                    b1col_sb[:],
                    ones_sb[:],
                    start=False,
                    stop=True,
                    skip_group_check=True,
                )
                lilj_tiles[t] = lilj_ps
                r_sb = rpool.tile([128, N], F16, tag="r", name="r")
                nc.scalar.copy(r_sb[:], lilj_ps[:, 0:N])
                s_sb = rpool.tile([128, K], F32, tag="s", name="s")
                nc.vector.tensor_copy(s_sb[:], lilj_ps[:, N : N + K])
                rs_tiles[t] = (r_sb, s_sb)
                if t % I_BATCH == 0:
                    ip_sb = iopool.tile(
                        [128, I_BATCH * 4 * N], F16, tag="ip", name="ip"
                    )
                    nc.sync.dma_start(
                        ip_sb[:],
                        ip_d[t : t + I_BATCH].rearrange("t p f -> p t f"),
                    )
                    ip_tiles[t // I_BATCH] = ip_sb

            def stage_b(t):
                lilj_ps = lilj_tiles.pop(t)
                r_sb, s_sb = rs_tiles.pop(t)
                hbufs = [
                    hidpool.tile([128, 4 * N], F16, tag=f"hb{w}", name=f"hb{w}")
                    for w in range(8)
                ]
                for g in range(K):
                    q, rem = divmod(g, 8)
                    h, c = divmod(rem, 4)
                    dst = hbufs[4 * h + q][:, c * N : (c + 1) * N]
                    s_col = s_sb[:, g : g + 1]
                    eng = _pw_engine(g)
                    if eng == "act":
                        nc.scalar.activation(
                            dst, r_sb[:], AFT.Relu, bias=s_col
                        )
                    elif eng == "pool":
                        nc.gpsimd.tensor_scalar(
                            dst, r_sb[:], s_col, 0.0, ALU.add, ALU.max
                        )
                    else:
                        nc.vector.tensor_scalar(
                            dst, r_sb[:], s_col, 0.0, ALU.add, ALU.max
                        )

                red_ps = ppool.tile(
                    [128, 4 * N], F32, tag="red", bufs=RED_BUFS, name="red"
                )
                for q in range(4):
                    nc.tensor.matmul(
                        red_ps[32 * q : 32 * q + 32, :],
                        wda_sb[:],
                        hbufs[q][:],
                        tile_position=(0, 32 * q),
                        start=True,
                        stop=False,
                        skip_group_check=True,
                    )
                    nc.tensor.matmul(
                        red_ps[32 * q : 32 * q + 32, :],
                        wdb_sb[:],
                        hbufs[4 + q][:],
                        tile_position=(0, 32 * q),
                        start=False,
                        stop=True,
                        skip_group_check=True,
                    )

                mi_sb = opool.tile([128, 8 * N], F16, tag="mi", name="mi")
                nc.scalar.activation(
                    mi_sb[:, 0 : 4 * N], red_ps[:], AFT.Sigmoid,
                    bias=b2col_sb[:, 0:1],
                )
                ip_sb = ip_tiles[t // I_BATCH]
                nc.vector.tensor_tensor(
                    mi_sb[:, 4 * N : 8 * N],
                    mi_sb[:, 0 : 4 * N],
                    ip_sb[:, (t % I_BATCH) * 4 * N : (t % I_BATCH + 1) * 4 * N],
                    ALU.mult,
                )
                nc.sync.dma_start(mi_d[t, :, :], mi_sb[:])

            SKEW = 2
            for t in range(min(SKEW, t_slices)):
                stage_a(t)
            for t in range(t_slices):
                stage_b(t)
                if t + SKEW < t_slices:
                    stage_a(t + SKEW)

    nc.compile()
    return nc


def make_aux_inputs(W1, b1, W2, b2):
    W1 = np.asarray(W1, np.float32)
    W1a = W1[:D]
    W1b = W1[D:]
    w1brep = np.zeros((D, 128), np.float16)
    for gp in range(4):
        w1brep[:, 32 * gp : 32 * gp + 32] = W1b.astype(np.float16)
    w1a = np.ascontiguousarray(W1a).astype(np.float16)
    b1col = np.tile(np.asarray(b1, np.float32), 4).reshape(1, 128).astype(np.float16)
    ones32 = np.ones((1, K), np.float16)
    # zero-padded block-diag W2 stationaries: phase h col (4h+m) has W2 at
    # partition block m
    wda = np.zeros((128, K), np.float16)
    wdb = np.zeros((128, K), np.float16)
    for m in range(4):
        wda[32 * m : 32 * m + 32, 4 * 0 + m] = np.asarray(W2, np.float16)
        wdb[32 * m : 32 * m + 32, 4 * 1 + m] = np.asarray(W2, np.float16)
    b2col = np.full((128, 1), np.asarray(b2, np.float32)[0], np.float32)
    return {
        "w1brep": w1brep,
        "w1a": w1a,
        "b1col": b1col,
        "ones32": ones32,
        "wda": wda,
        "wdb": wdb,
        "b2col": b2col,
    }


def _perm_maps():
    """row r, chunk c -> i mapping of the permuted output/I layout.

    r = 32q + 4h + m (valid for r%32 < 8), i = 32m + 8q + 4h + c.
    Returns (valid_rows[32], i_of[32, 4]) where valid_rows are the 32 rows
    holding data and i_of gives i per (row_idx, chunk).
    """
    rows = []
    i_of = []
    for q in range(4):
        for h in range(2):
            for m in range(4):
                r = 32 * q + 4 * h + m
                rows.append(r)
                i_of.append([32 * m + 8 * q + 4 * h + c for c in range(4)])
    return np.array(rows), np.array(i_of)


ROWS, I_OF = _perm_maps()


def permute_i(ifull_core):
    """I_full (T, N, N) f32 -> permuted fp16 (T, N, 4N) matching the
    on-device layout; junk rows left zero."""
    out = np.zeros((T, 128, 4 * N), np.float16)
    src = ifull_core.astype(np.float16)
    for ridx, r in enumerate(ROWS):
        for c in range(4):
            out[:, r, c * N : (c + 1) * N] = src[:, I_OF[ridx, c], :]
    return out


def unpermute(mi_core):
    """Permuted (T, N, 8N) fp16 -> (M, I_sparse) each (T, N, N) f32."""
    M = np.empty((T, N, N), np.float32)
    Isp = np.empty((T, N, N), np.float32)
    for ridx, r in enumerate(ROWS):
        for c in range(4):
            i = I_OF[ridx, c]
            M[:, i, :] = mi_core[:, r, c * N : (c + 1) * N].astype(np.float32)
            Isp[:, i, :] = mi_core[:, r, 4 * N + c * N : 4 * N + (c + 1) * N].astype(
                np.float32
            )
    return Isp, M


TRACE = False
LAST_RESULTS = None


@functools.lru_cache(maxsize=1)
def _built_nc():
    return _build(T)


def kernel(**inputs):
    from concourse.bass_utils import run_bass_kernel_spmd

    h = np.asarray(inputs["h"])
    # (B, T, N, D) -> (B, D, T*N) so one DMA per core loads all hT
    ht = np.ascontiguousarray(
        np.transpose(h, (0, 3, 1, 2)).reshape(B, D, -1)
    ).astype(np.float16)
    ifull = np.asarray(inputs["I_full"], np.float32)
    aux = make_aux_inputs(
        inputs["W1"], inputs["b1"], inputs["W2"], inputs["b2"]
    )

    nc = _built_nc()
    in_maps = [
        {"ht": ht[cc], "ip": permute_i(ifull[cc]), **aux} for cc in range(NCORES)
    ]
    res = run_bass_kernel_spmd(
        nc, in_maps, core_ids=list(range(NCORES)), trace=TRACE
    )
    global LAST_RESULTS
    LAST_RESULTS = res
    isp = np.empty((B, T, N, N), np.float32)
    m = np.empty((B, T, N, N), np.float32)
    for cc in range(NCORES):
        i_c, m_c = unpermute(res.results[cc]["mi"])
        isp[cc] = i_c
        m[cc] = m_c
    return isp, m


# revision 7
# speedup vs baseline: 1.1140x; 1.0001x over previous
"""Trainium2 Bass kernel for the EdgeMask problem.

Computes, for h (B,T,N,d), I_full (B,T,N,N), MLP params W1 (2d,hid) b1 (hid,)
W2 (hid,) b2 (1,):
    li = h @ W1[:d]; lj = h @ W1[d:]
    hid = relu(li[:,:,:,None,:] + lj[:,:,None,:,:] + b1)
    M = sigmoid(hid @ W2 + b2);  I_sparse = I_full * M
Returns (I_sparse, M).

Sharding: data-parallel over B across 8 NeuronCores (B=8), no collectives.

Per-core pipeline (per t slice, N=128, d=128, K=32 hidden):
  - PE: ljT-replicated (one matmul, W1b pre-replicated 4x in cols) and a
    li "stack" S[32*gp+k, g] = li[g+32*gp, k] (+b1 via a rank-1 accumulate
    matmul), both in one PSUM tile.
  - Pointwise hid_g = relu(R + S[:, g]) as 32 fp16 tensor_scalar ops
    ([128,128] each, bias read straight from PSUM), split across
    DVE / ACT / GPSIMD.
  - Reduce over k on PE: per column strip q, two accumulating matmuls with
    zero-padded block-diag W2 stationaries (phase h in {0,1}) consume the
    8 hid buffers; result is a COMPACT [128, 512] PSUM tile whose rows
    32q+4h+m hold logits for i = 32m+8q+4h+c at free chunk c.
  - ACT applies sigmoid(+b2) on the full [128,512] tile (junk rows incl.),
    DVE multiplies with the host-prepermuted I tile, both halves land in
    one [128, 1024] fp16 tile, stored permuted; the host unpermutes/casts.
"""

import functools

import numpy as np

import concourse.bass as bass
import concourse.mybir as mybir
import concourse.tile as tile
from concourse import bacc

F32 = mybir.dt.float32
F16 = mybir.dt.float16

B = 8
T = 32
N = 128
D = 128
K = 32  # hidden
NCORES = 8

AFT = mybir.ActivationFunctionType
ALU = mybir.AluOpType

# pointwise split: group g -> engine. roughly DVE 19 / ACT 6 / Pool 7
PW_DVE = 19
PW_ACT = 5
PW_POOL = 8
HID_BUFS = 2
IO_BUFS = 2
OUT_BUFS = 3
R_BUFS = 4
LILJ_BUFS = 3
RED_BUFS = 2
I_BATCH = 4  # slices per I-load DMA
R_ON_POOL = False


def _pw_engine(g):
    # deterministic interleave so each engine's work spreads over the slice
    seq = (["dve"] * PW_DVE + ["act"] * PW_ACT + ["pool"] * PW_POOL)
    return seq[(g * 7) % K]


def _build(t_slices: int = T):
    nc = bacc.Bacc(
        "TRN2", target_bir_lowering=False, debug=False, num_devices=NCORES
    )

    ht_d = nc.dram_tensor("ht", [D, t_slices * N], F16, kind="ExternalInput")
    ip_d = nc.dram_tensor("ip", [t_slices, N, 4 * N], F16, kind="ExternalInput")
    w1brep_d = nc.dram_tensor("w1brep", [D, 128], F16, kind="ExternalInput")
    w1a_d = nc.dram_tensor("w1a", [D, K], F16, kind="ExternalInput")
    b1col_d = nc.dram_tensor("b1col", [1, 128], F16, kind="ExternalInput")
    ones_d = nc.dram_tensor("ones32", [1, K], F16, kind="ExternalInput")
    wda_d = nc.dram_tensor("wda", [128, K], F16, kind="ExternalInput")
    wdb_d = nc.dram_tensor("wdb", [128, K], F16, kind="ExternalInput")
    b2col_d = nc.dram_tensor("b2col", [128, 1], F32, kind="ExternalInput")

    # permuted merged output: [..., 0:512] = M, [..., 512:1024] = I_sparse
    mi_d = nc.dram_tensor("mi", [t_slices, N, 8 * N], F16, kind="ExternalOutput")

    with tile.TileContext(nc) as tc:
        with (
            tc.tile_pool(name="const", bufs=1) as cpool,
            tc.tile_pool(name="rsb", bufs=R_BUFS) as rpool,
            tc.tile_pool(name="hid", bufs=HID_BUFS) as hidpool,
            tc.tile_pool(name="io", bufs=IO_BUFS) as iopool,
            tc.tile_pool(name="outp", bufs=OUT_BUFS) as opool,
            tc.tile_pool(name="psum", bufs=1, space="PSUM") as ppool,
        ):
            w1brep_sb = cpool.tile([D, 128], F16)
            nc.sync.dma_start(w1brep_sb[:], w1brep_d[:])
            w1a_sb = cpool.tile([D, K], F16)
            nc.sync.dma_start(w1a_sb[:], w1a_d[:])
            b1col_sb = cpool.tile([1, 128], F16)
            nc.sync.dma_start(b1col_sb[:], b1col_d[:])
            ones_sb = cpool.tile([1, K], F16)
            nc.sync.dma_start(ones_sb[:], ones_d[:])
            wda_sb = cpool.tile([128, K], F16)
            nc.sync.dma_start(wda_sb[:], wda_d[:])
            wdb_sb = cpool.tile([128, K], F16)
            nc.sync.dma_start(wdb_sb[:], wdb_d[:])
            b2col_sb = cpool.tile([128, 1], F32)
            nc.sync.dma_start(b2col_sb[:], b2col_d[:])

            # all slices' hT, chunked so slice 0 can start early
            htall_sb = cpool.tile([D, t_slices * N], F16)
            n_chunks = min(8, t_slices)
            chunk = t_slices * N // n_chunks
            for ci in range(n_chunks):
                nc.sync.dma_start(
                    htall_sb[:, ci * chunk : (ci + 1) * chunk],
                    ht_d[:, ci * chunk : (ci + 1) * chunk],
                )

            lilj_tiles = {}
            rs_tiles = {}
            ip_tiles = {}

            def stage_a(t):
                ht_sb = htall_sb[:, t * N : (t + 1) * N]
                lilj_ps = ppool.tile(
                    [128, N + K], F32, tag="lilj", bufs=LILJ_BUFS, name="lilj"
                )
                nc.tensor.matmul(lilj_ps[:, 0:N], w1brep_sb[:], ht_sb)
                for gp in range(4):
                    nc.tensor.matmul(
                        lilj_ps[32 * gp : 32 * gp + 32, N : N + K],
                        w1a_sb[:],
                        ht_sb[:, 32 * gp : 32 * gp + 32],
                        tile_position=(0, 32 * gp),
                        start=True,
                        stop=False,
                        skip_group_check=True,
                    )
                nc.tensor.matmul(
                    lilj_ps[:, N : N + K],
                    b1col_sb[:],
                    ones_sb[:],
                    start=False,
                    stop=True,
                    skip_group_check=True,
                )
                lilj_tiles[t] = lilj_ps
                r_sb = rpool.tile([128, N], F16, tag="r", name="r")
                nc.scalar.copy(r_sb[:], lilj_ps[:, 0:N])
                s_sb = rpool.tile([128, K], F32, tag="s", name="s")
                nc.vector.tensor_copy(s_sb[:], lilj_ps[:, N : N + K])
                rs_tiles[t] = (r_sb, s_sb)
                if t % I_BATCH == 0:
                    ip_sb = iopool.tile(
                        [128, I_BATCH * 4 * N], F16, tag="ip", name="ip"
                    )
                    nc.sync.dma_start(
                        ip_sb[:],
                        ip_d[t : t + I_BATCH].rearrange("t p f -> p t f"),
                    )
                    ip_tiles[t // I_BATCH] = ip_sb

            def stage_b(t):
                lilj_ps = lilj_tiles.pop(t)
                r_sb, s_sb = rs_tiles.pop(t)
                hbufs = [
                    hidpool.tile([128, 4 * N], F16, tag=f"hb{w}", name=f"hb{w}")
                    for w in range(8)
                ]
                for g in range(K):
                    q, rem = divmod(g, 8)
                    h, c = divmod(rem, 4)
                    dst = hbufs[4 * h + q][:, c * N : (c + 1) * N]
                    s_col = s_sb[:, g : g + 1]
                    eng = _pw_engine(g)
                    if eng == "act":
                        nc.scalar.activation(
                            dst, r_sb[:], AFT.Relu, bias=s_col
                        )
                    elif eng == "pool":
                        nc.gpsimd.tensor_scalar(
                            dst, r_sb[:], s_col, 0.0, ALU.add, ALU.max
                        )
                    else:
                        nc.vector.tensor_scalar(
                            dst, r_sb[:], s_col, 0.0, ALU.add, ALU.max
                        )

                red_ps = ppool.tile(
                    [128, 4 * N], F32, tag="red", bufs=RED_BUFS, name="red"
                )
                for q in range(4):
                    nc.tensor.matmul(
                        red_ps[32 * q : 32 * q + 32, :],
                        wda_sb[:],
                        hbufs[q][:],
                        tile_position=(0, 32 * q),
                        start=True,
                        stop=False,
                        skip_group_check=True,
                    )
                    nc.tensor.matmul(
                        red_ps[32 * q : 32 * q + 32, :],
                        wdb_sb[:],
                        hbufs[4 + q][:],
                        tile_position=(0, 32 * q),
                        start=False,
                        stop=True,
                        skip_group_check=True,
                    )

                mi_sb = opool.tile([128, 8 * N], F16, tag="mi", name="mi")
                nc.scalar.activation(
                    mi_sb[:, 0 : 4 * N], red_ps[:], AFT.Sigmoid,
                    bias=b2col_sb[:, 0:1],
                )
                ip_sb = ip_tiles[t // I_BATCH]
                nc.vector.tensor_tensor(
                    mi_sb[:, 4 * N : 8 * N],
                    mi_sb[:, 0 : 4 * N],
                    ip_sb[:, (t % I_BATCH) * 4 * N : (t % I_BATCH + 1) * 4 * N],
                    ALU.mult,
                )
                nc.sync.dma_start(mi_d[t, :, :], mi_sb[:])

            SKEW = 2
            for t in range(min(SKEW, t_slices)):
                stage_a(t)
            for t in range(t_slices):
                stage_b(t)
                if t + SKEW < t_slices:
                    stage_a(t + SKEW)

    nc.compile()
    return nc


def make_aux_inputs(W1, b1, W2, b2):
    W1 = np.asarray(W1, np.float32)
    W1a = W1[:D]
    W1b = W1[D:]
    w1brep = np.zeros((D, 128), np.float16)
    for gp in range(4):
        w1brep[:, 32 * gp : 32 * gp + 32] = W1b.astype(np.float16)
    w1a = np.ascontiguousarray(W1a).astype(np.float16)
    b1col = np.tile(np.asarray(b1, np.float32), 4).reshape(1, 128).astype(np.float16)
    ones32 = np.ones((1, K), np.float16)
    # zero-padded block-diag W2 stationaries: phase h col (4h+m) has W2 at
    # partition block m
    wda = np.zeros((128, K), np.float16)
    wdb = np.zeros((128, K), np.float16)
    for m in range(4):
        wda[32 * m : 32 * m + 32, 4 * 0 + m] = np.asarray(W2, np.float16)
        wdb[32 * m : 32 * m + 32, 4 * 1 + m] = np.asarray(W2, np.float16)
    b2col = np.full((128, 1), np.asarray(b2, np.float32)[0], np.float32)
    return {
        "w1brep": w1brep,
        "w1a": w1a,
        "b1col": b1col,
        "ones32": ones32,
        "wda": wda,
        "wdb": wdb,
        "b2col": b2col,
    }


def _perm_maps():
    """row r, chunk c -> i mapping of the permuted output/I layout.

    r = 32q + 4h + m (valid for r%32 < 8), i = 32m + 8q + 4h + c.
    Returns (valid_rows[32], i_of[32, 4]) where valid_rows are the 32 rows
    holding data and i_of gives i per (row_idx, chunk).
    """
    rows = []
    i_of = []
    for q in range(4):
        for h in range(2):
            for m in range(4):
                r = 32 * q + 4 * h + m
                rows.append(r)
                i_of.append([32 * m + 8 * q + 4 * h + c for c in range(4)])
    return np.array(rows), np.array(i_of)


ROWS, I_OF = _perm_maps()


def permute_i(ifull_core):
    """I_full (T, N, N) f32 -> permuted fp16 (T, N, 4N) matching the
    on-device layout; junk rows left zero."""
    out = np.zeros((T, 128, 4 * N), np.float16)
    src = ifull_core.astype(np.float16)
    for ridx, r in enumerate(ROWS):
        for c in range(4):
            out[:, r, c * N : (c + 1) * N] = src[:, I_OF[ridx, c], :]
    return out


def unpermute(mi_core):
    """Permuted (T, N, 8N) fp16 -> (M, I_sparse) each (T, N, N) f32."""
    M = np.empty((T, N, N), np.float32)
    Isp = np.empty((T, N, N), np.float32)
    for ridx, r in enumerate(ROWS):
        for c in range(4):
            i = I_OF[ridx, c]
            M[:, i, :] = mi_core[:, r, c * N : (c + 1) * N].astype(np.float32)
            Isp[:, i, :] = mi_core[:, r, 4 * N + c * N : 4 * N + (c + 1) * N].astype(
                np.float32
            )
    return Isp, M


TRACE = False
LAST_RESULTS = None


@functools.lru_cache(maxsize=1)
def _built_nc():
    return _build(T)


def kernel(**inputs):
    from concourse.bass_utils import run_bass_kernel_spmd

    h = np.asarray(inputs["h"])
    # (B, T, N, D) -> (B, D, T*N) so one DMA per core loads all hT
    ht = np.ascontiguousarray(
        np.transpose(h, (0, 3, 1, 2)).reshape(B, D, -1)
    ).astype(np.float16)
    ifull = np.asarray(inputs["I_full"], np.float32)
    aux = make_aux_inputs(
        inputs["W1"], inputs["b1"], inputs["W2"], inputs["b2"]
    )

    nc = _built_nc()
    in_maps = [
        {"ht": ht[cc], "ip": permute_i(ifull[cc]), **aux} for cc in range(NCORES)
    ]
    res = run_bass_kernel_spmd(
        nc, in_maps, core_ids=list(range(NCORES)), trace=TRACE
    )
    global LAST_RESULTS
    LAST_RESULTS = res
    isp = np.empty((B, T, N, N), np.float32)
    m = np.empty((B, T, N, N), np.float32)
    for cc in range(NCORES):
        i_c, m_c = unpermute(res.results[cc]["mi"])
        isp[cc] = i_c
        m[cc] = m_c
    return isp, m


# revision 9
# speedup vs baseline: 1.1908x; 1.0690x over previous
"""Trainium2 Bass kernel for the EdgeMask problem.

Computes, for h (B,T,N,d), I_full (B,T,N,N), MLP params W1 (2d,hid) b1 (hid,)
W2 (hid,) b2 (1,):
    li = h @ W1[:d]; lj = h @ W1[d:]
    hid = relu(li[:,:,:,None,:] + lj[:,:,None,:,:] + b1)
    M = sigmoid(hid @ W2 + b2);  I_sparse = I_full * M
Returns (I_sparse, M).

Sharding: data-parallel over B across 8 NeuronCores (B=8), no collectives.

Per-core pipeline (per t slice, N=128, d=128, K=32 hidden):
  - PE: ljT-replicated (one matmul, W1b pre-replicated 4x in cols) and a
    li "stack" S[32*gp+k, g] = li[g+32*gp, k] (+b1 via a rank-1 accumulate
    matmul), both in one PSUM tile.
  - Pointwise hid_g = relu(R + S[:, g]) as 32 fp16 tensor_scalar ops
    ([128,128] each, bias read straight from PSUM), split across
    DVE / ACT / GPSIMD.
  - Reduce over k on PE: per column strip q, two accumulating matmuls with
    zero-padded block-diag W2 stationaries (phase h in {0,1}) consume the
    8 hid buffers; result is a COMPACT [128, 512] PSUM tile whose rows
    32q+4h+m hold logits for i = 32m+8q+4h+c at free chunk c.
  - ACT applies sigmoid(+b2) on the full [128,512] tile (junk rows incl.),
    DVE multiplies with the host-prepermuted I tile, both halves land in
    one [128, 1024] fp16 tile, stored permuted; the host unpermutes/casts.
"""

import functools

import numpy as np

import concourse.bass as bass
import concourse.mybir as mybir
import concourse.tile as tile
from concourse import bacc

F32 = mybir.dt.float32
F16 = mybir.dt.float16

B = 8
T = 32
N = 128
D = 128
K = 32  # hidden
NCORES = 8

AFT = mybir.ActivationFunctionType
ALU = mybir.AluOpType

# pointwise split: group g -> engine. roughly DVE 19 / ACT 6 / Pool 7
PW_DVE = 19
PW_ACT = 5
PW_POOL = 8
HID_BUFS = 2
IO_BUFS = 2
OUT_BUFS = 3
R_BUFS = 4
LILJ_BUFS = 3
RED_BUFS = 2
I_BATCH = 4  # slices per I-load DMA
R_ON_POOL = False


def _pw_engine(g):
    # deterministic interleave so each engine's work spreads over the slice
    seq = (["dve"] * PW_DVE + ["act"] * PW_ACT + ["pool"] * PW_POOL)
    return seq[(g * 7) % K]


def _build(t_slices: int = T):
    nc = bacc.Bacc(
        "TRN2", target_bir_lowering=False, debug=False, num_devices=NCORES
    )

    ht_d = nc.dram_tensor("ht", [D, t_slices * N], F16, kind="ExternalInput")
    ip_d = nc.dram_tensor("ip", [t_slices, N, 2 * N], F16, kind="ExternalInput")
    w1brep_d = nc.dram_tensor("w1brep", [D, 128], F16, kind="ExternalInput")
    w1a_d = nc.dram_tensor("w1a", [D, K], F16, kind="ExternalInput")
    b1col_d = nc.dram_tensor("b1col", [1, 128], F16, kind="ExternalInput")
    ones_d = nc.dram_tensor("ones32", [1, K], F16, kind="ExternalInput")
    wd_d = nc.dram_tensor("wd", [4, 128, 32], F16, kind="ExternalInput")
    b2col_d = nc.dram_tensor("b2col", [128, 1], F32, kind="ExternalInput")

    # permuted merged output: [..., 0:512] = M, [..., 512:1024] = I_sparse
    mi_d = nc.dram_tensor("mi", [t_slices, N, 4 * N], F16, kind="ExternalOutput")

    with tile.TileContext(nc) as tc:
        with (
            tc.tile_pool(name="const", bufs=1) as cpool,
            tc.tile_pool(name="rsb", bufs=R_BUFS) as rpool,
            tc.tile_pool(name="hid", bufs=HID_BUFS) as hidpool,
            tc.tile_pool(name="io", bufs=IO_BUFS) as iopool,
            tc.tile_pool(name="outp", bufs=OUT_BUFS) as opool,
            tc.tile_pool(name="psum", bufs=1, space="PSUM") as ppool,
        ):
            w1brep_sb = cpool.tile([D, 128], F16)
            nc.sync.dma_start(w1brep_sb[:], w1brep_d[:])
            w1a_sb = cpool.tile([D, K], F16)
            nc.sync.dma_start(w1a_sb[:], w1a_d[:])
            b1col_sb = cpool.tile([1, 128], F16)
            nc.sync.dma_start(b1col_sb[:], b1col_d[:])
            ones_sb = cpool.tile([1, K], F16)
            nc.sync.dma_start(ones_sb[:], ones_d[:])
            wd_sbs = []
            for p in range(4):
                wd_sb = cpool.tile([128, 32], F16, tag=f"wd{p}", name=f"wd{p}")
                nc.sync.dma_start(wd_sb[:], wd_d[p])
                wd_sbs.append(wd_sb)
            b2col_sb = cpool.tile([128, 1], F32)
            nc.sync.dma_start(b2col_sb[:], b2col_d[:])

            # all slices' hT, chunked so slice 0 can start early
            htall_sb = cpool.tile([D, t_slices * N], F16)
            n_chunks = min(8, t_slices)
            chunk = t_slices * N // n_chunks
            for ci in range(n_chunks):
                nc.sync.dma_start(
                    htall_sb[:, ci * chunk : (ci + 1) * chunk],
                    ht_d[:, ci * chunk : (ci + 1) * chunk],
                )

            lilj_tiles = {}
            rs_tiles = {}
            ip_tiles = {}
            red_tiles = {}

            def stage_a(t):
                ht_sb = htall_sb[:, t * N : (t + 1) * N]
                lilj_ps = ppool.tile(
                    [128, N + K], F32, tag="lilj", bufs=LILJ_BUFS, name="lilj"
                )
                nc.tensor.matmul(lilj_ps[:, 0:N], w1brep_sb[:], ht_sb)
                for gp in range(4):
                    nc.tensor.matmul(
                        lilj_ps[32 * gp : 32 * gp + 32, N : N + K],
                        w1a_sb[:],
                        ht_sb[:, 32 * gp : 32 * gp + 32],
                        tile_position=(0, 32 * gp),
                        start=True,
                        stop=False,
                        skip_group_check=True,
                    )
                nc.tensor.matmul(
                    lilj_ps[:, N : N + K],
                    b1col_sb[:],
                    ones_sb[:],
                    start=False,
                    stop=True,
                    skip_group_check=True,
                )
                lilj_tiles[t] = lilj_ps
                r_sb = rpool.tile([128, N], F16, tag="r", name="r")
                nc.scalar.copy(r_sb[:], lilj_ps[:, 0:N])
                s_sb = rpool.tile([128, K], F32, tag="s", name="s")
                nc.vector.tensor_copy(s_sb[:], lilj_ps[:, N : N + K])
                rs_tiles[t] = (r_sb, s_sb)
                if t % I_BATCH == 0:
                    ip_sb = iopool.tile(
                        [128, I_BATCH * 2 * N], F16, tag="ip", name="ip"
                    )
                    nc.sync.dma_start(
                        ip_sb[:],
                        ip_d[t : t + I_BATCH].rearrange("t p f -> p t f"),
                    )
                    ip_tiles[t // I_BATCH] = ip_sb

            def stage_b(t):
                lilj_ps = lilj_tiles.pop(t)
                r_sb, s_sb = rs_tiles.pop(t)
                hbufs = [
                    hidpool.tile([128, 2 * N], F16, tag=f"hb{w}", name=f"hb{w}")
                    for w in range(16)
                ]
                for g in range(K):
                    p, rem = divmod(g, 8)
                    q, c = divmod(rem, 2)
                    dst = hbufs[4 * p + q][:, c * N : (c + 1) * N]
                    s_col = s_sb[:, g : g + 1]
                    eng = _pw_engine(g)
                    if eng == "act":
                        nc.scalar.activation(
                            dst, r_sb[:], AFT.Relu, bias=s_col
                        )
                    elif eng == "pool":
                        nc.gpsimd.tensor_scalar(
                            dst, r_sb[:], s_col, 0.0, ALU.add, ALU.max
                        )
                    else:
                        nc.vector.tensor_scalar(
                            dst, r_sb[:], s_col, 0.0, ALU.add, ALU.max
                        )

                red_ps = ppool.tile(
                    [128, 2 * N], F32, tag="red", bufs=RED_BUFS, name="red"
                )
                for q in range(4):
                    for p in range(4):
                        nc.tensor.matmul(
                            red_ps[32 * q : 32 * q + 32, :],
                            wd_sbs[p][:],
                            hbufs[4 * p + q][:],
                            tile_position=(0, 32 * q),
                            start=(p == 0),
                            stop=(p == 3),
                            skip_group_check=True,
                        )
                red_tiles[t] = red_ps

            def stage_c(t):
                red_ps = red_tiles.pop(t)
                mi_sb = opool.tile([128, 4 * N], F16, tag="mi", name="mi")
                nc.scalar.activation(
                    mi_sb[:, 0 : 2 * N], red_ps[:], AFT.Sigmoid,
                    bias=b2col_sb[:, 0:1],
                )
                ip_sb = ip_tiles[t // I_BATCH]
                nc.vector.tensor_tensor(
                    mi_sb[:, 2 * N : 4 * N],
                    mi_sb[:, 0 : 2 * N],
                    ip_sb[:, (t % I_BATCH) * 2 * N : (t % I_BATCH + 1) * 2 * N],
                    ALU.mult,
                )
                nc.sync.dma_start(mi_d[t, :, :], mi_sb[:])

            SKEW = 2
            for t in range(min(SKEW, t_slices)):
                stage_a(t)
            for t in range(t_slices):
                if t >= 1:
                    stage_c(t - 1)
                stage_b(t)
                if t + SKEW < t_slices:
                    stage_a(t + SKEW)
            stage_c(t_slices - 1)

    nc.compile()
    return nc


def make_aux_inputs(W1, b1, W2, b2):
    W1 = np.asarray(W1, np.float32)
    W1a = W1[:D]
    W1b = W1[D:]
    w1brep = np.zeros((D, 128), np.float16)
    for gp in range(4):
        w1brep[:, 32 * gp : 32 * gp + 32] = W1b.astype(np.float16)
    w1a = np.ascontiguousarray(W1a).astype(np.float16)
    b1col = np.tile(np.asarray(b1, np.float32), 4).reshape(1, 128).astype(np.float16)
    ones32 = np.ones((1, K), np.float16)
    # 4-phase zero-padded block-diag W2 stationaries: phase p col (4p+m)
    # carries W2 at partition block m
    wd = np.zeros((4, 128, 32), np.float16)
    for p in range(4):
        for m in range(4):
            wd[p, 32 * m : 32 * m + 32, 4 * p + m] = np.asarray(W2, np.float16)
    b2col = np.full((128, 1), np.asarray(b2, np.float32)[0], np.float32)
    return {
        "w1brep": w1brep,
        "w1a": w1a,
        "b1col": b1col,
        "ones32": ones32,
        "wd": wd,
        "b2col": b2col,
    }


def _perm_maps():
    """row r = 32q+4p+m (valid for r%32 < 16), chunk c in {0,1} ->
    i = 32m + 8p + 2q + c."""
    rows = []
    i_of = []
    for q in range(4):
        for p in range(4):
            for m in range(4):
                r = 32 * q + 4 * p + m
                rows.append(r)
                i_of.append([32 * m + 8 * p + 2 * q + c for c in range(2)])
    return np.array(rows), np.array(i_of)


ROWS, I_OF = _perm_maps()


def permute_i(ifull_core):
    """I_full (T, N, N) f32 -> permuted fp16 (T, N, 2N) matching the
    on-device layout; junk rows left zero."""
    out = np.zeros((T, 128, 2 * N), np.float16)
    src = ifull_core.astype(np.float16)
    for ridx, r in enumerate(ROWS):
        for c in range(2):
            out[:, r, c * N : (c + 1) * N] = src[:, I_OF[ridx, c], :]
    return out


def unpermute(mi_core):
    """Permuted (T, N, 4N) fp16 -> (I_sparse, M) each (T, N, N) f32."""
    M = np.empty((T, N, N), np.float32)
    Isp = np.empty((T, N, N), np.float32)
    for ridx, r in enumerate(ROWS):
        for c in range(2):
            i = I_OF[ridx, c]
            M[:, i, :] = mi_core[:, r, c * N : (c + 1) * N].astype(np.float32)
            Isp[:, i, :] = mi_core[:, r, 2 * N + c * N : 2 * N + (c + 1) * N].astype(
                np.float32
            )
    return Isp, M


TRACE = False
LAST_RESULTS = None


@functools.lru_cache(maxsize=1)
def _built_nc():
    return _build(T)


def kernel(**inputs):
    from concourse.bass_utils import run_bass_kernel_spmd

    h = np.asarray(inputs["h"])
    # (B, T, N, D) -> (B, D, T*N) so one DMA per core loads all hT
    ht = np.ascontiguousarray(
        np.transpose(h, (0, 3, 1, 2)).reshape(B, D, -1)
    ).astype(np.float16)
    ifull = np.asarray(inputs["I_full"], np.float32)
    aux = make_aux_inputs(
        inputs["W1"], inputs["b1"], inputs["W2"], inputs["b2"]
    )

    nc = _built_nc()
    in_maps = [
        {"ht": ht[cc], "ip": permute_i(ifull[cc]), **aux} for cc in range(NCORES)
    ]
    res = run_bass_kernel_spmd(
        nc, in_maps, core_ids=list(range(NCORES)), trace=TRACE
    )
    global LAST_RESULTS
    LAST_RESULTS = res
    isp = np.empty((B, T, N, N), np.float32)
    m = np.empty((B, T, N, N), np.float32)
    for cc in range(NCORES):
        i_c, m_c = unpermute(res.results[cc]["mi"])
        isp[cc] = i_c
        m[cc] = m_c
    return isp, m


# revision 10
# speedup vs baseline: 1.2676x; 1.0645x over previous
"""Trainium2 Bass kernel for the EdgeMask problem.

Computes, for h (B,T,N,d), I_full (B,T,N,N), MLP params W1 (2d,hid) b1 (hid,)
W2 (hid,) b2 (1,):
    li = h @ W1[:d]; lj = h @ W1[d:]
    hid = relu(li[:,:,:,None,:] + lj[:,:,None,:,:] + b1)
    M = sigmoid(hid @ W2 + b2);  I_sparse = I_full * M
Returns (I_sparse, M).

Sharding: data-parallel over B across 8 NeuronCores (B=8), no collectives.

Per-core pipeline (per t slice, N=128, d=128, K=32 hidden):
  - PE: ljT-replicated (one matmul, W1b pre-replicated 4x in cols) and a
    li "stack" S[32*gp+k, g] = li[g+32*gp, k] (+b1 via a rank-1 accumulate
    matmul), both in one PSUM tile.
  - Pointwise hid_g = relu(R + S[:, g]) as 32 fp16 tensor_scalar ops
    ([128,128] each, bias read straight from PSUM), split across
    DVE / ACT / GPSIMD.
  - Reduce over k on PE: per column strip q, two accumulating matmuls with
    zero-padded block-diag W2 stationaries (phase h in {0,1}) consume the
    8 hid buffers; result is a COMPACT [128, 512] PSUM tile whose rows
    32q+4h+m hold logits for i = 32m+8q+4h+c at free chunk c.
  - ACT applies sigmoid(+b2) on the full [128,512] tile (junk rows incl.),
    DVE multiplies with the host-prepermuted I tile, both halves land in
    one [128, 1024] fp16 tile, stored permuted; the host unpermutes/casts.
"""

import functools

import numpy as np

import concourse.bass as bass
import concourse.mybir as mybir
import concourse.tile as tile
from concourse import bacc

F32 = mybir.dt.float32
F16 = mybir.dt.float16

B = 8
T = 32
N = 128
D = 128
K = 32  # hidden
NCORES = 8

AFT = mybir.ActivationFunctionType
ALU = mybir.AluOpType

# pointwise split: group g -> engine. roughly DVE 19 / ACT 6 / Pool 7
PW_DVE = 19
PW_ACT = 5
PW_POOL = 8
HID_BUFS = 2
IO_BUFS = 2
OUT_BUFS = 3
R_BUFS = 4
LILJ_BUFS = 3
RED_BUFS = 2
I_BATCH = 4  # slices per I-load DMA
R_ON_POOL = False


def _pw_engine(g):
    # deterministic interleave so each engine's work spreads over the slice
    seq = (["dve"] * PW_DVE + ["act"] * PW_ACT + ["pool"] * PW_POOL)
    return seq[(g * 7) % K]


def _build(t_slices: int = T):
    nc = bacc.Bacc(
        "TRN2", target_bir_lowering=False, debug=False, num_devices=NCORES
    )

    ht_d = nc.dram_tensor("ht", [D, t_slices * N], F16, kind="ExternalInput")
    ip_d = nc.dram_tensor("ip", [t_slices, N, 2 * N], F16, kind="ExternalInput")
    blob_d = nc.dram_tensor("blob", [D, 416], F16, kind="ExternalInput")
    b2col_d = nc.dram_tensor("b2col", [128, 1], F32, kind="ExternalInput")

    # permuted merged output: [..., 0:512] = M, [..., 512:1024] = I_sparse
    mi_d = nc.dram_tensor("mi", [t_slices, N, 4 * N], F16, kind="ExternalOutput")

    with tile.TileContext(nc) as tc:
        with (
            tc.tile_pool(name="const", bufs=1) as cpool,
            tc.tile_pool(name="rsb", bufs=R_BUFS) as rpool,
            tc.tile_pool(name="hid", bufs=HID_BUFS) as hidpool,
            tc.tile_pool(name="io", bufs=IO_BUFS) as iopool,
            tc.tile_pool(name="outp", bufs=OUT_BUFS) as opool,
            tc.tile_pool(name="psum", bufs=1, space="PSUM") as ppool,
        ):
            # first ht chunk before everything else so slice 0 starts early
            n_chunks = min(8, t_slices)
            chunk = t_slices * N // n_chunks
            htall_sb = cpool.tile([D, t_slices * N], F16)
            nc.sync.dma_start(htall_sb[:, 0:chunk], ht_d[:, 0:chunk])

            blob_sb = cpool.tile([D, 416], F16)
            nc.sync.dma_start(blob_sb[:], blob_d[:])
            w1brep_sb = blob_sb[:, 0:128]
            w1a_sb = blob_sb[:, 128:160]
            wd_sbs = [blob_sb[:, 160 + 32 * p : 192 + 32 * p] for p in range(4)]
            b1col_sb = blob_sb[0:1, 288:416]
            ones_sb = cpool.tile([1, K], F16)
            nc.vector.memset(ones_sb[:], 1)
            b2col_sb = cpool.tile([128, 1], F32)
            nc.sync.dma_start(b2col_sb[:], b2col_d[:])
            for ci in range(1, n_chunks):
                nc.sync.dma_start(
                    htall_sb[:, ci * chunk : (ci + 1) * chunk],
                    ht_d[:, ci * chunk : (ci + 1) * chunk],
                )

            lilj_tiles = {}
            rs_tiles = {}
            ip_tiles = {}
            red_tiles = {}

            def stage_a(t):
                ht_sb = htall_sb[:, t * N : (t + 1) * N]
                lilj_ps = ppool.tile(
                    [128, N + K], F32, tag="lilj", bufs=LILJ_BUFS, name="lilj"
                )
                nc.tensor.matmul(lilj_ps[:, 0:N], w1brep_sb[:], ht_sb)
                for gp in range(4):
                    nc.tensor.matmul(
                        lilj_ps[32 * gp : 32 * gp + 32, N : N + K],
                        w1a_sb[:],
                        ht_sb[:, 32 * gp : 32 * gp + 32],
                        tile_position=(0, 32 * gp),
                        start=True,
                        stop=False,
                        skip_group_check=True,
                    )
                nc.tensor.matmul(
                    lilj_ps[:, N : N + K],
                    b1col_sb[:],
                    ones_sb[:],
                    start=False,
                    stop=True,
                    skip_group_check=True,
                )
                lilj_tiles[t] = lilj_ps
                r_sb = rpool.tile([128, N], F16, tag="r", name="r")
                nc.scalar.copy(r_sb[:], lilj_ps[:, 0:N])
                s_sb = rpool.tile([128, K], F32, tag="s", name="s")
                nc.vector.tensor_copy(s_sb[:], lilj_ps[:, N : N + K])
                rs_tiles[t] = (r_sb, s_sb)
                if t % I_BATCH == 0:
                    ip_sb = iopool.tile(
                        [128, I_BATCH * 2 * N], F16, tag="ip", name="ip"
                    )
                    nc.sync.dma_start(
                        ip_sb[:],
                        ip_d[t : t + I_BATCH].rearrange("t p f -> p t f"),
                    )
                    ip_tiles[t // I_BATCH] = ip_sb

            def stage_b(t):
                lilj_ps = lilj_tiles.pop(t)
                r_sb, s_sb = rs_tiles.pop(t)
                hbufs = [
                    hidpool.tile([128, 2 * N], F16, tag=f"hb{w}", name=f"hb{w}")
                    for w in range(16)
                ]
                for g in range(K):
                    p, rem = divmod(g, 8)
                    q, c = divmod(rem, 2)
                    dst = hbufs[4 * p + q][:, c * N : (c + 1) * N]
                    s_col = s_sb[:, g : g + 1]
                    eng = _pw_engine(g)
                    if eng == "act":
                        nc.scalar.activation(
                            dst, r_sb[:], AFT.Relu, bias=s_col
                        )
                    elif eng == "pool":
                        nc.gpsimd.tensor_scalar(
                            dst, r_sb[:], s_col, 0.0, ALU.add, ALU.max
                        )
                    else:
                        nc.vector.tensor_scalar(
                            dst, r_sb[:], s_col, 0.0, ALU.add, ALU.max
                        )

                red_ps = ppool.tile(
                    [128, 2 * N], F32, tag="red", bufs=RED_BUFS, name="red"
                )
                for q in range(4):
                    for p in range(4):
                        nc.tensor.matmul(
                            red_ps[32 * q : 32 * q + 32, :],
                            wd_sbs[p][:],
                            hbufs[4 * p + q][:],
                            tile_position=(0, 32 * q),
                            start=(p == 0),
                            stop=(p == 3),
                            skip_group_check=True,
                        )
                red_tiles[t] = red_ps

            def stage_c(t):
                red_ps = red_tiles.pop(t)
                mi_sb = opool.tile([128, 4 * N], F16, tag="mi", name="mi")
                nc.scalar.activation(
                    mi_sb[:, 0 : 2 * N], red_ps[:], AFT.Sigmoid,
                    bias=b2col_sb[:, 0:1],
                )
                ip_sb = ip_tiles[t // I_BATCH]
                nc.vector.tensor_tensor(
                    mi_sb[:, 2 * N : 4 * N],
                    mi_sb[:, 0 : 2 * N],
                    ip_sb[:, (t % I_BATCH) * 2 * N : (t % I_BATCH + 1) * 2 * N],
                    ALU.mult,
                )
                nc.sync.dma_start(mi_d[t, :, :], mi_sb[:])

            SKEW = 2
            for t in range(min(SKEW, t_slices)):
                stage_a(t)
            for t in range(t_slices):
                if t >= 1:
                    stage_c(t - 1)
                stage_b(t)
                if t + SKEW < t_slices:
                    stage_a(t + SKEW)
            stage_c(t_slices - 1)

    nc.compile()
    return nc


def make_aux_inputs(W1, b1, W2, b2):
    W1 = np.asarray(W1, np.float32)
    W1a = W1[:D]
    W1b = W1[D:]
    blob = np.zeros((D, 416), np.float16)
    for gp in range(4):
        blob[:, 32 * gp : 32 * gp + 32] = W1b.astype(np.float16)
    blob[:, 128:160] = W1a.astype(np.float16)
    # 4-phase zero-padded block-diag W2: phase p col (4p+m) has W2 at block m
    for p in range(4):
        for m in range(4):
            blob[32 * m : 32 * m + 32, 160 + 32 * p + 4 * p + m] = np.asarray(
                W2, np.float16
            )
    blob[0, 288:416] = np.tile(np.asarray(b1, np.float32), 4).astype(np.float16)
    b2col = np.full((128, 1), np.asarray(b2, np.float32)[0], np.float32)
    return {
        "blob": blob,
        "b2col": b2col,
    }


def _perm_maps():
    """row r = 32q+4p+m (valid for r%32 < 16), chunk c in {0,1} ->
    i = 32m + 8p + 2q + c."""
    rows = []
    i_of = []
    for q in range(4):
        for p in range(4):
            for m in range(4):
                r = 32 * q + 4 * p + m
                rows.append(r)
                i_of.append([32 * m + 8 * p + 2 * q + c for c in range(2)])
    return np.array(rows), np.array(i_of)


ROWS, I_OF = _perm_maps()


def permute_i(ifull_core):
    """I_full (T, N, N) f32 -> permuted fp16 (T, N, 2N) matching the
    on-device layout; junk rows left zero."""
    out = np.zeros((T, 128, 2 * N), np.float16)
    src = ifull_core.astype(np.float16)
    for ridx, r in enumerate(ROWS):
        for c in range(2):
            out[:, r, c * N : (c + 1) * N] = src[:, I_OF[ridx, c], :]
    return out


def unpermute(mi_core):
    """Permuted (T, N, 4N) fp16 -> (I_sparse, M) each (T, N, N) f32."""
    M = np.empty((T, N, N), np.float32)
    Isp = np.empty((T, N, N), np.float32)
    for ridx, r in enumerate(ROWS):
        for c in range(2):
            i = I_OF[ridx, c]
            M[:, i, :] = mi_core[:, r, c * N : (c + 1) * N].astype(np.float32)
            Isp[:, i, :] = mi_core[:, r, 2 * N + c * N : 2 * N + (c + 1) * N].astype(
                np.float32
            )
    return Isp, M


TRACE = False
LAST_RESULTS = None


@functools.lru_cache(maxsize=1)
def _built_nc():
    return _build(T)


def kernel(**inputs):
    from concourse.bass_utils import run_bass_kernel_spmd

    h = np.asarray(inputs["h"])
    # (B, T, N, D) -> (B, D, T*N) so one DMA per core loads all hT
    ht = np.ascontiguousarray(
        np.transpose(h, (0, 3, 1, 2)).reshape(B, D, -1)
    ).astype(np.float16)
    ifull = np.asarray(inputs["I_full"], np.float32)
    aux = make_aux_inputs(
        inputs["W1"], inputs["b1"], inputs["W2"], inputs["b2"]
    )

    nc = _built_nc()
    in_maps = [
        {"ht": ht[cc], "ip": permute_i(ifull[cc]), **aux} for cc in range(NCORES)
    ]
    res = run_bass_kernel_spmd(
        nc, in_maps, core_ids=list(range(NCORES)), trace=TRACE
    )
    global LAST_RESULTS
    LAST_RESULTS = res
    isp = np.empty((B, T, N, N), np.float32)
    m = np.empty((B, T, N, N), np.float32)
    for cc in range(NCORES):
        i_c, m_c = unpermute(res.results[cc]["mi"])
        isp[cc] = i_c
        m[cc] = m_c
    return isp, m
